# revision 38
# baseline (speedup 1.0000x reference)
"""Multi-head self-attention (no mask) on 8 TRN2 NeuronCores — v4.

Problem: B=2, T=2048, C=1024, H=16 heads, D=64.
    q/k/v = x @ W{q,k,v}.T + b;  att = softmax(q k^T / sqrt(D));
    y = att v;  out = y @ Wp.T + bp.

Sharding: core (b, g) = batch x head-group of 4 heads.  Each core computes
q/k/v for its 4 heads, attention, and the partial output projection through
its 256 columns of Wp; the host sums 4 partials per batch and adds bp.

v4 changes vs v3 (288us -> 223us):
  - Unified PSUM ring: one tag of [128, 1024] fp32 tiles (bufs=3, 6 banks)
    shared by S tiles, q/k projections, v projection and the out-
    projection; py0/py1 accumulators single-buffered (2 banks). All 8
    banks accounted; no cross-pool coupling stalls. (Matmul streams stay
    <=512 wide - ISA cap of one PSUM bank - but bias-adds/copies/exp span
    the full 1024-wide tiles.)
  - Fast normalize: py -> SBUF copies FIRST (release both PSUM banks
    ~1us after the s-loop); 1/sigma via reciprocal_approx_fast at
    partition 0 (the custom DVE op is only correct at base partition 0;
    exact reciprocal on one partition costs 3.3us).
  - Depth-2 S prefetch: two exp tiles in flight so Y(s) never waits ACT.
  - Independent-chain interleave: the two 512-halves of each projection
    (different PSUM banks) alternate, removing same-bank accumulation
    serialization (v-proj runs back-to-back at pure stream rate).
  - Few large strictly-need-ordered input DMAs (the DMA channel is
    serial with a cumulative completion semaphore), x block 0 first.
  - Minimal prologue (k/q for t 0:512 only) so the ACT exp stream starts
    ~10us earlier; everything else is JIT extras in the s-loops.
  - bf16 partial outputs (host sums in f32), halved out-DMA drain.
  - Fused tail: last chunk normalizes in 128-column blocks, each out-
    projection launched as soon as its yT block lands; its ob copies/DMAs
    split across DVE+sync and ACT (idle after the exp stream ends).
  - S matmuls remain emitted as base-partition 0/64 pairs (the PE runs
    the two 64-row quadrant matmuls concurrently).
"""

import sys
from contextlib import ExitStack

import numpy as np
import ml_dtypes

if "/opt/trn_rl_repo" not in sys.path:
    sys.path.insert(0, "/opt/trn_rl_repo")

import concourse.bass as bass  # noqa: F401
import concourse.mybir as mybir
import concourse.tile as tile
from concourse import bacc
from concourse.bass_utils import run_bass_kernel_spmd

F32 = mybir.dt.float32
BF16 = mybir.dt.bfloat16
Act = mybir.ActivationFunctionType

NP_BF16 = ml_dtypes.bfloat16

P = 128
B, C, HEADS, D = 2, 1024, 16, 64
GROUPS = 4            # head groups (one per core within a batch)
HLOC = HEADS // GROUPS
G = HLOC * D          # 256 channels per core
KT = C // P           # 8 contraction chunks
VW = D + 1            # v + ones column
LN16 = float(np.log(16.0))


def build(T=2048):
    TQ = 512              # attention query chunk (psum-bank limited)
    TW = 1024             # projection moving-stream width
    NTQ = T // TQ         # 4 query chunks
    NS = T // P           # 16 key tiles
    NTW = T // TW         # 2 projection chunks

    nc = bacc.Bacc("TRN2", target_bir_lowering=False, debug=False)
    xT = nc.dram_tensor("xt", [C, T], BF16, kind="ExternalInput")
    wq = nc.dram_tensor("wq", [C, G], BF16, kind="ExternalInput")
    wk = nc.dram_tensor("wk", [C, G], BF16, kind="ExternalInput")
    wv = nc.dram_tensor("wv", [C, G], BF16, kind="ExternalInput")
    wp = nc.dram_tensor("wp", [G, C], BF16, kind="ExternalInput")
    bq = nc.dram_tensor("bq", [G], F32, kind="ExternalInput")
    bk = nc.dram_tensor("bk", [G], F32, kind="ExternalInput")
    bv = nc.dram_tensor("bv", [G], BF16, kind="ExternalInput")
    out = nc.dram_tensor("out", [T, C], BF16, kind="ExternalOutput")

    with tile.TileContext(nc) as tc, ExitStack() as ctx:
        persist = ctx.enter_context(tc.tile_pool(name="persist", bufs=1))

        xs = persist.tile([P, KT, T], BF16, tag="xs")
        wq_sb = persist.tile([P, KT, G], BF16, tag="wq_sb")
        wk_sb = persist.tile([P, KT, G], BF16, tag="wk_sb")
        wv_sb = persist.tile([P, KT, G], BF16, tag="wv_sb")
        wp_sb = persist.tile([P, 2, C], BF16, tag="wp_sb")
        bq_pp = persist.tile([P, 2], F32, tag="bq_pp")
        bk_pp = persist.tile([P, 2], F32, tag="bk_pp")
        bv_row = persist.tile([1, G], BF16, tag="bv_row")
        ones_col = persist.tile([1, P], BF16, tag="ones_col")
        expb = persist.tile([P, 1], F32, tag="expb")
        qT = persist.tile([P, 2, T], BF16, tag="qT")
        kT = persist.tile([P, 2, T], BF16, tag="kT")
        v_sb = persist.tile([P, NS, HLOC, VW], BF16, tag="v_sb")
        yT = persist.tile([P, 2, T], BF16, tag="yT")

        nc.gpsimd.memset(ones_col[:], 1.0)
        nc.gpsimd.memset(expb[:], -LN16)
        nc.gpsimd.memset(v_sb[:, :, :, D : D + 1], 1.0)  # just the ones col

        # --- input DMAs: weights for head-pair 0 first (they gate the first
        # projections), then x per contraction chunk, then the rest
        # Few LARGE DMAs, ordered by first use: the DMA channel is serial and
        # the completion semaphore is cumulative, so many small DMAs (or any
        # out-of-order DMA) delay every consumer behind them.
        def dma_x(tb):
            ts = slice(tb * 512, (tb + 1) * 512)
            nc.sync.dma_start(
                xs[:, :, ts], xT[:, ts].rearrange("(k p) t -> p k t", p=P)
            )

        dma_x(0)
        nc.sync.dma_start(wk_sb[:], wk[:, :].rearrange("(k p) g -> p k g", p=P))
        nc.sync.dma_start(wq_sb[:], wq[:, :].rearrange("(k p) g -> p k g", p=P))
        nc.sync.dma_start(wv_sb[:], wv[:, :].rearrange("(k p) g -> p k g", p=P))
        dma_x(1)
        nc.sync.dma_start(bq_pp[:], bq[:].rearrange("(m p) -> p m", p=P))
        nc.sync.dma_start(bk_pp[:], bk[:].rearrange("(m p) -> p m", p=P))
        nc.sync.dma_start(bv_row[:], bv[None, :])
        dma_x(2)
        dma_x(3)
        nc.sync.dma_start(wp_sb[:], wp[:, :].rearrange("(m p) c -> p m c", p=P))

        with (
            tc.tile_pool(name="ring", bufs=3, space="PSUM") as ring,
            tc.tile_pool(name="psY", bufs=1, space="PSUM") as psY,
            tc.tile_pool(name="ptp", bufs=3) as ptp,
            tc.tile_pool(name="npool", bufs=2) as npool,
            tc.tile_pool(name="obuf", bufs=2) as obuf,
        ):
            def proj_qk(m, cw, which):
                """q or k projection: dst[:, m, cw*TW:(cw+1)*TW].  Matmul
                streams are ISA-capped at 512 elements (one PSUM bank), so
                the two 512 halves go into one ring tile and share a single
                1024-wide DVE bias-add."""
                w_sb, b_pp, dst = (
                    (wq_sb, bq_pp, qT) if which == 0 else (wk_sb, bk_pp, kT)
                )
                ts = slice(cw * TW, (cw + 1) * TW)
                pq = ring.tile([P, TW], F32, tag="big", name="pq")
                # interleave the two independent 512-halves (different PSUM
                # banks) so adjacent matmuls pipeline without the same-bank
                # accumulation serialization
                for kk in range(KT):
                    for half in range(2):
                        hs = slice(half * 512, (half + 1) * 512)
                        xts = slice(
                            cw * TW + half * 512, cw * TW + (half + 1) * 512
                        )
                        nc.tensor.matmul(
                            pq[:, hs],
                            w_sb[:, kk, m * P : (m + 1) * P],
                            xs[:, kk, xts],
                            start=(kk == 0),
                            stop=(kk == KT - 1),
                        )
                nc.vector.tensor_scalar_add(dst[:, m, ts], pq[:], b_pp[:, m : m + 1])

            def proj_qk_half(m, cw, half, which):
                """single 512-wide q/k chunk — used in the prologue so the
                s-loop can start as soon as one query chunk is projected."""
                w_sb, b_pp, dst = (
                    (wq_sb, bq_pp, qT) if which == 0 else (wk_sb, bk_pp, kT)
                )
                ts = slice(cw * TW + half * 512, cw * TW + (half + 1) * 512)
                pq = ring.tile([P, TW], F32, tag="big", name="pqh")
                for kk in range(KT):
                    nc.tensor.matmul(
                        pq[:, 0:512],
                        w_sb[:, kk, m * P : (m + 1) * P],
                        xs[:, kk, ts],
                        start=(kk == 0),
                        stop=(kk == KT - 1),
                    )
                nc.vector.tensor_scalar_add(
                    dst[:, m, ts], pq[:, 0:512], b_pp[:, m : m + 1]
                )

            def proj_v(u):
                """v for key tiles 2u, 2u+1 -> v_sb.  The two tiles go to
                different PSUM banks of one ring tile with their kk chains
                interleaved, so adjacent matmuls are independent."""
                pv = ring.tile([P, TW], F32, tag="big")
                regions = (slice(0, G), slice(512, 512 + G))
                for kk in range(KT):
                    for i in range(2):
                        nc.tensor.matmul(
                            pv[:, regions[i]],
                            xs[:, kk, (2 * u + i) * P : (2 * u + i + 1) * P],
                            wv_sb[:, kk, :],
                            start=(kk == 0),
                            stop=False,
                            skip_group_check=True,
                        )
                for i in range(2):
                    nc.tensor.matmul(
                        pv[:, regions[i]], ones_col[0:1, :], bv_row[0:1, :],
                        start=False, stop=True,
                        skip_group_check=True,
                    )
                for i in range(2):
                    nc.vector.tensor_copy(
                        v_sb[:, 2 * u + i, :, 0:D],
                        pv[:, regions[i]].rearrange("p (h d) -> p h d", d=D),
                    )

            def oproj(mt, act_copy=False):
                po = ring.tile([P, TW], F32, tag="big", name="po")
                for j in range(2):  # halves interleaved: independent banks
                    for n in range(2):
                        ns = slice(n * 512, (n + 1) * 512)
                        nc.tensor.matmul(
                            po[:, ns],
                            yT[:, j, mt * P : (mt + 1) * P],
                            wp_sb[:, j, ns],
                            start=(j == 0),
                            stop=(j == 1),
                        )
                ob = obuf.tile([P, C], BF16, tag="ob")
                for n in range(2):  # halves: DMA overlaps the second copy;
                    # in the tail the exp stream is done, so ACT takes one
                    # half end-to-end (its own copy AND its own DMA channel)
                    ns = slice(n * 512, (n + 1) * 512)
                    if act_copy and n == 0:
                        nc.scalar.copy(ob[:, ns], po[:, ns])
                        nc.scalar.dma_start(
                            out[mt * P : (mt + 1) * P, ns], ob[:, ns]
                        )
                    else:
                        nc.vector.tensor_copy(ob[:, ns], po[:, ns])
                        nc.sync.dma_start(
                            out[mt * P : (mt + 1) * P, ns], ob[:, ns]
                        )

            def s_mm(pi, tq, s):
                """S scores for key tile s, both heads of pair pi (quadrant
                pair: base partitions 0 and 64 run concurrently on the PE)."""
                ts = slice(tq * TQ, (tq + 1) * TQ)
                sp = ring.tile([P, 2 * TQ], F32, tag="big")
                for hh in range(2):
                    bp_ = 64 * hh
                    nc.tensor.matmul(
                        sp[:, hh * TQ : (hh + 1) * TQ],
                        kT[bp_ : bp_ + 64, pi, s * P : (s + 1) * P],
                        qT[bp_ : bp_ + 64, pi, ts],
                        start=True,
                        stop=True,
                    )
                # P = exp(S/8 - ln16) in bf16 (the shift cancels in y'/sigma)
                pt = ptp.tile([P, 2, TQ], BF16, tag="pt")
                nc.scalar.activation(
                    pt[:], sp[:], Act.Exp, bias=expb[:, 0:1], scale=0.125,
                )
                return pt

            def normalize_tail(pi, tq):
                """Last chunk: normalize in 128-column blocks and launch each
                out-projection as soon as its block of yT is ready, so the PE
                restarts early and the copies/DMAs pipeline behind it."""
                ts = slice(tq * TQ, (tq + 1) * TQ)
                y_uns, bcasts = [], []
                for hh in range(2):
                    py = (py0_ref[0], py1_ref[0])[hh]
                    y_un = npool.tile([VW, TQ], F32, tag=f"yun{hh}")
                    nc.vector.tensor_copy(y_un[:], py[:])
                    y_uns.append(y_un)
                for hh in range(2):
                    rec0 = npool.tile([1, TQ], F32, tag=f"rec0{hh}")
                    nc.sync.dma_start(rec0[:], y_uns[hh][D : D + 1, :])
                    rinv = npool.tile([1, TQ], F32, tag=f"rinv{hh}")
                    nc.vector.reciprocal_approx_fast(rinv[0:1, :], rec0[0:1, :])
                    bcast = npool.tile([D, TQ], F32, tag=f"bcast{hh}")
                    nc.gpsimd.partition_broadcast(
                        bcast[:, :], rinv[0:1, :], channels=D
                    )
                    bcasts.append(bcast)
                y_tmp = npool.tile([D, TQ], BF16, tag="y_tmp")
                for w in range(4):
                    bs = slice(w * P, (w + 1) * P)
                    tbs = slice(tq * TQ + w * P, tq * TQ + (w + 1) * P)
                    nc.vector.tensor_mul(
                        yT[0:D, pi, tbs], y_uns[0][0:D, bs], bcasts[0][:, bs]
                    )
                    nc.vector.tensor_mul(
                        y_tmp[:, bs], y_uns[1][0:D, bs], bcasts[1][:, bs]
                    )
                    nc.sync.dma_start(yT[D : 2 * D, pi, tbs], y_tmp[:, bs])
                    oproj(tq * 4 + w, act_copy=True)

            def normalize(pi, tq):
                """y_h <- y_h / sigma_h for both heads of pair pi, query
                chunk tq.  Copies py to SBUF first (releases the PSUM bank
                ~1us after the s-loop), then the slow part runs on
                DVE/Pool/DMA off the PE critical path."""
                ts = slice(tq * TQ, (tq + 1) * TQ)
                y_uns = []
                for hh in range(2):  # both copies first: frees py banks fast
                    py = (py0_ref[0], py1_ref[0])[hh]
                    y_un = npool.tile([VW, TQ], F32, tag=f"yun{hh}")
                    nc.vector.tensor_copy(y_un[:], py[:])
                    y_uns.append(y_un)
                for hh in range(2):
                    y_un = y_uns[hh]
                    # sigma to partition 0 (reciprocal_approx_fast is only
                    # correct at base partition 0), approx 1/sigma, broadcast
                    rec0 = npool.tile([1, TQ], F32, tag=f"rec0{hh}")
                    nc.sync.dma_start(rec0[:], y_un[D : D + 1, :])
                    rinv = npool.tile([1, TQ], F32, tag=f"rinv{hh}")
                    nc.vector.reciprocal_approx_fast(rinv[0:1, :], rec0[0:1, :])
                    bcast = npool.tile([D, TQ], F32, tag=f"bcast{hh}")
                    nc.gpsimd.partition_broadcast(
                        bcast[:, :], rinv[0:1, :], channels=D
                    )
                    if hh == 0:
                        nc.vector.tensor_mul(
                            yT[0:D, pi, ts], y_un[0:D, :], bcast[:, :]
                        )
                    else:
                        y_tmp = npool.tile([D, TQ], BF16, tag="y_tmp")
                        nc.vector.tensor_mul(y_tmp[:], y_un[0:D, :], bcast[:, :])
                        nc.sync.dma_start(yT[D : 2 * D, pi, ts], y_tmp[:])

            # JIT emission schedule: extras[(pi, tq, s)] = thunks run at the
            # top of that attention s iteration (PE program order).
            extras = {}
            # v lumps lead their Y consumers by >=2 iterations; v(0) is NOT
            # in the prologue — its wv/x DMAs land ~10us in, and emitting it
            # before S(0) would stall the in-order PE queue on the DMA
            vsched = {0: 1, 1: 2, 2: 3, 3: 4, 4: 6, 5: 8, 6: 10, 7: 12}
            for u, key in vsched.items():
                extras.setdefault((0, 0, key), []).append(
                    lambda u=u: proj_v(u)
                )
            # second halves of k/q chunk 0 (prologue did only t 0:512)
            extras.setdefault((0, 0, 1), []).append(
                lambda: proj_qk_half(0, 0, 1, 1)
            )
            extras.setdefault((0, 0, 5), []).append(
                lambda: proj_qk_half(0, 0, 1, 0)
            )
            # spread the remaining projection lumps into the otherwise
            # ACT-bound later chunks (each lump is needed >=1 chunk later
            # than its slot): the PE fills its exp-wait instead of idling
            extras.setdefault((0, 0, 3), []).append(lambda: proj_qk(0, 1, 1))
            extras.setdefault((0, 1, 0), []).append(lambda: proj_qk(0, 1, 0))
            extras.setdefault((0, 2, 0), []).append(lambda: proj_qk(1, 0, 1))
            extras.setdefault((0, 3, 0), []).append(lambda: proj_qk(1, 1, 1))
            extras.setdefault((0, 3, 8), []).append(lambda: proj_qk(1, 0, 0))
            extras.setdefault((1, 0, 0), []).append(lambda: proj_qk(1, 1, 0))
            for tq in range(1, NTQ):  # deferred out-projection of tq-1
                for w, key in enumerate((6, 9, 11, 13)):
                    extras.setdefault((1, tq, key), []).append(
                        lambda tq=tq, w=w: oproj((tq - 1) * 4 + w)
                    )

            # minimal prologue: k and q for t 0:512 only — the s-loop (and
            # with it the ACT exp stream) starts as soon as those project
            proj_qk_half(0, 0, 0, 1)
            proj_qk_half(0, 0, 0, 0)

            py0_ref = [None]
            py1_ref = [None]

            for pi in range(2):
                for tq in range(NTQ):
                    py0_ref[0] = psY.tile(
                        [VW, TQ], F32, tag="py0", name="py0"
                    )
                    py1_ref[0] = psY.tile(
                        [VW, TQ], F32, tag="py1", name="py1"
                    )
                    pts = {}
                    for j in range(2):  # depth-2 S prefetch: two exp tiles
                        for fn in extras.get((pi, tq, j), ()):  # in flight
                            fn()
                        pts[j] = s_mm(pi, tq, j)
                    for s in range(NS):
                        if s + 2 < NS:
                            for fn in extras.get((pi, tq, s + 2), ()):
                                fn()
                            pts[s + 2] = s_mm(pi, tq, s + 2)
                        pt = pts.pop(s)
                        for hh in range(2):
                            h = 2 * pi + hh
                            nc.tensor.matmul(
                                (py0_ref[0], py1_ref[0])[hh][:],
                                v_sb[:, s, h, :],
                                pt[:, hh, :],
                                start=(s == 0),
                                stop=(s == NS - 1),
                            )
                    if pi == 1 and tq == NTQ - 1:
                        normalize_tail(pi, tq)
                    else:
                        normalize(pi, tq)

    nc.finalize()
    return nc


_NC_CACHE = {}


def _get_nc(T=2048):
    if T not in _NC_CACHE:
        _NC_CACHE[T] = build(T=T)
    return _NC_CACHE[T]


def _make_in_maps(x, Wq, bq, Wk, bk, Wv, bv, Wp):
    in_maps = []
    for b in range(B):
        xt = np.ascontiguousarray(x[b].T).astype(NP_BF16)
        for g in range(GROUPS):
            sl = slice(g * G, (g + 1) * G)
            in_maps.append(
                {
                    "xt": xt,
                    "wq": np.ascontiguousarray(Wq[sl, :].T).astype(NP_BF16),
                    "wk": np.ascontiguousarray(Wk[sl, :].T).astype(NP_BF16),
                    "wv": np.ascontiguousarray(Wv[sl, :].T).astype(NP_BF16),
                    "wp": np.ascontiguousarray(Wp[:, sl].T).astype(NP_BF16),
                    "bq": np.ascontiguousarray(bq[sl], dtype=np.float32),
                    "bk": np.ascontiguousarray(bk[sl], dtype=np.float32),
                    "bv": np.ascontiguousarray(bv[sl]).astype(NP_BF16),
                }
            )
    return in_maps


def run(inputs, trace=False):
    """Run on 8 cores; returns (out [B,T,C] fp32, BassKernelResults)."""
    x = np.asarray(inputs["x"], dtype=np.float32)
    T = x.shape[1]
    in_maps = _make_in_maps(
        x,
        np.asarray(inputs["Wq"]), np.asarray(inputs["bq"]),
        np.asarray(inputs["Wk"]), np.asarray(inputs["bk"]),
        np.asarray(inputs["Wv"]), np.asarray(inputs["bv"]),
        np.asarray(inputs["Wp"]),
    )
    nc = _get_nc(T)
    res = run_bass_kernel_spmd(
        nc, in_maps, core_ids=list(range(B * GROUPS)), trace=trace
    )
    bp = np.asarray(inputs["bp"], dtype=np.float32)
    parts = [
        res.results[i]["out"].astype(np.float32) for i in range(B * GROUPS)
    ]
    out = np.stack(
        [sum(parts[b * GROUPS : (b + 1) * GROUPS]) for b in range(B)]
    ) + bp[None, None, :]
    return out.astype(np.float32), res


def kernel(**inputs):
    out, _ = run(inputs, trace=False)
    return out


# revision 39
# speedup vs baseline: 1.0088x; 1.0088x over previous
"""Multi-head self-attention (no mask) on 8 TRN2 NeuronCores — v4.

Problem: B=2, T=2048, C=1024, H=16 heads, D=64.
    q/k/v = x @ W{q,k,v}.T + b;  att = softmax(q k^T / sqrt(D));
    y = att v;  out = y @ Wp.T + bp.

Sharding: core (b, g) = batch x head-group of 4 heads.  Each core computes
q/k/v for its 4 heads, attention, and the partial output projection through
its 256 columns of Wp; the host sums 4 partials per batch and adds bp.

v4 changes vs v3 (288us -> 223us):
  - Unified PSUM ring: one tag of [128, 1024] fp32 tiles (bufs=3, 6 banks)
    shared by S tiles, q/k projections, v projection and the out-
    projection; py0/py1 accumulators single-buffered (2 banks). All 8
    banks accounted; no cross-pool coupling stalls. (Matmul streams stay
    <=512 wide - ISA cap of one PSUM bank - but bias-adds/copies/exp span
    the full 1024-wide tiles.)
  - Fast normalize: py -> SBUF copies FIRST (release both PSUM banks
    ~1us after the s-loop); 1/sigma via reciprocal_approx_fast at
    partition 0 (the custom DVE op is only correct at base partition 0;
    exact reciprocal on one partition costs 3.3us).
  - Depth-2 S prefetch: two exp tiles in flight so Y(s) never waits ACT.
  - Independent-chain interleave: the two 512-halves of each projection
    (different PSUM banks) alternate, removing same-bank accumulation
    serialization (v-proj runs back-to-back at pure stream rate).
  - Few large strictly-need-ordered input DMAs (the DMA channel is
    serial with a cumulative completion semaphore), x block 0 first.
  - Minimal prologue (k/q for t 0:512 only) so the ACT exp stream starts
    ~10us earlier; everything else is JIT extras in the s-loops.
  - bf16 partial outputs (host sums in f32), halved out-DMA drain.
  - Fused tail: last chunk normalizes in 128-column blocks, each out-
    projection launched as soon as its yT block lands; its ob copies/DMAs
    split across DVE+sync and ACT (idle after the exp stream ends).
  - S matmuls remain emitted as base-partition 0/64 pairs (the PE runs
    the two 64-row quadrant matmuls concurrently).
"""

import sys
from contextlib import ExitStack

import numpy as np
import ml_dtypes

if "/opt/trn_rl_repo" not in sys.path:
    sys.path.insert(0, "/opt/trn_rl_repo")

import concourse.bass as bass  # noqa: F401
import concourse.mybir as mybir
import concourse.tile as tile
from concourse import bacc
from concourse.bass_utils import run_bass_kernel_spmd

F32 = mybir.dt.float32
BF16 = mybir.dt.bfloat16
Act = mybir.ActivationFunctionType

NP_BF16 = ml_dtypes.bfloat16

P = 128
B, C, HEADS, D = 2, 1024, 16, 64
GROUPS = 4            # head groups (one per core within a batch)
HLOC = HEADS // GROUPS
G = HLOC * D          # 256 channels per core
KT = C // P           # 8 contraction chunks
VW = D + 1            # v + ones column
LN16 = float(np.log(16.0))


def build(T=2048):
    TQ = 512              # attention query chunk (psum-bank limited)
    TW = 1024             # projection moving-stream width
    NTQ = T // TQ         # 4 query chunks
    NS = T // P           # 16 key tiles
    NTW = T // TW         # 2 projection chunks

    nc = bacc.Bacc("TRN2", target_bir_lowering=False, debug=False)
    xT = nc.dram_tensor("xt", [C, T], BF16, kind="ExternalInput")
    wq = nc.dram_tensor("wq", [C, G], BF16, kind="ExternalInput")
    wk = nc.dram_tensor("wk", [C, G], BF16, kind="ExternalInput")
    wv = nc.dram_tensor("wv", [C, G], BF16, kind="ExternalInput")
    wp = nc.dram_tensor("wp", [G, C], BF16, kind="ExternalInput")
    bq = nc.dram_tensor("bq", [G], F32, kind="ExternalInput")
    bk = nc.dram_tensor("bk", [G], F32, kind="ExternalInput")
    bv = nc.dram_tensor("bv", [G], BF16, kind="ExternalInput")
    out = nc.dram_tensor("out", [T, C], BF16, kind="ExternalOutput")

    with tile.TileContext(nc) as tc, ExitStack() as ctx:
        persist = ctx.enter_context(tc.tile_pool(name="persist", bufs=1))

        xs = persist.tile([P, KT, T], BF16, tag="xs")
        wq_sb = persist.tile([P, KT, G], BF16, tag="wq_sb")
        wk_sb = persist.tile([P, KT, G], BF16, tag="wk_sb")
        wv_sb = persist.tile([P, KT, G], BF16, tag="wv_sb")
        wp_sb = persist.tile([P, 2, C], BF16, tag="wp_sb")
        bq_pp = persist.tile([P, 2], F32, tag="bq_pp")
        bk_pp = persist.tile([P, 2], F32, tag="bk_pp")
        bv_row = persist.tile([1, G], BF16, tag="bv_row")
        ones_col = persist.tile([1, P], BF16, tag="ones_col")
        expb = persist.tile([P, 1], F32, tag="expb")
        qT = persist.tile([P, 2, T], BF16, tag="qT")
        kT = persist.tile([P, 2, T], BF16, tag="kT")
        v_sb = persist.tile([P, NS, HLOC, VW], BF16, tag="v_sb")
        yT = persist.tile([P, 2, T], BF16, tag="yT")

        nc.gpsimd.memset(ones_col[:], 1.0)
        nc.gpsimd.memset(expb[:], -LN16)
        nc.gpsimd.memset(v_sb[:, :, :, D : D + 1], 1.0)  # just the ones col

        # --- input DMAs: weights for head-pair 0 first (they gate the first
        # projections), then x per contraction chunk, then the rest
        # Few LARGE DMAs, ordered by first use: the DMA channel is serial and
        # the completion semaphore is cumulative, so many small DMAs (or any
        # out-of-order DMA) delay every consumer behind them.
        def dma_x(tb):
            ts = slice(tb * 512, (tb + 1) * 512)
            nc.sync.dma_start(
                xs[:, :, ts], xT[:, ts].rearrange("(k p) t -> p k t", p=P)
            )

        dma_x(0)
        nc.sync.dma_start(wk_sb[:], wk[:, :].rearrange("(k p) g -> p k g", p=P))
        nc.sync.dma_start(wq_sb[:], wq[:, :].rearrange("(k p) g -> p k g", p=P))
        nc.sync.dma_start(wv_sb[:], wv[:, :].rearrange("(k p) g -> p k g", p=P))
        dma_x(1)
        nc.sync.dma_start(bq_pp[:], bq[:].rearrange("(m p) -> p m", p=P))
        nc.sync.dma_start(bk_pp[:], bk[:].rearrange("(m p) -> p m", p=P))
        nc.sync.dma_start(bv_row[:], bv[None, :])
        dma_x(2)
        dma_x(3)
        nc.sync.dma_start(wp_sb[:], wp[:, :].rearrange("(m p) c -> p m c", p=P))

        with (
            tc.tile_pool(name="ring", bufs=3, space="PSUM") as ring,
            tc.tile_pool(name="psY", bufs=1, space="PSUM") as psY,
            tc.tile_pool(name="ptp", bufs=3) as ptp,
            tc.tile_pool(name="npool", bufs=2) as npool,
            tc.tile_pool(name="obuf", bufs=2) as obuf,
        ):
            def proj_qk(m, cw, which):
                """q or k projection: dst[:, m, cw*TW:(cw+1)*TW].  Matmul
                streams are ISA-capped at 512 elements (one PSUM bank), so
                the two 512 halves go into one ring tile and share a single
                1024-wide DVE bias-add."""
                w_sb, b_pp, dst = (
                    (wq_sb, bq_pp, qT) if which == 0 else (wk_sb, bk_pp, kT)
                )
                ts = slice(cw * TW, (cw + 1) * TW)
                pq = ring.tile([P, TW], F32, tag="big", name="pq")
                # interleave the two independent 512-halves (different PSUM
                # banks) so adjacent matmuls pipeline without the same-bank
                # accumulation serialization
                for kk in range(KT):
                    for half in range(2):
                        hs = slice(half * 512, (half + 1) * 512)
                        xts = slice(
                            cw * TW + half * 512, cw * TW + (half + 1) * 512
                        )
                        nc.tensor.matmul(
                            pq[:, hs],
                            w_sb[:, kk, m * P : (m + 1) * P],
                            xs[:, kk, xts],
                            start=(kk == 0),
                            stop=(kk == KT - 1),
                        )
                nc.vector.tensor_scalar_add(dst[:, m, ts], pq[:], b_pp[:, m : m + 1])

            def proj_qk_half(m, cw, half, which):
                """single 512-wide q/k chunk — used in the prologue so the
                s-loop can start as soon as one query chunk is projected."""
                w_sb, b_pp, dst = (
                    (wq_sb, bq_pp, qT) if which == 0 else (wk_sb, bk_pp, kT)
                )
                ts = slice(cw * TW + half * 512, cw * TW + (half + 1) * 512)
                pq = ring.tile([P, TW], F32, tag="big", name="pqh")
                for kk in range(KT):
                    nc.tensor.matmul(
                        pq[:, 0:512],
                        w_sb[:, kk, m * P : (m + 1) * P],
                        xs[:, kk, ts],
                        start=(kk == 0),
                        stop=(kk == KT - 1),
                    )
                nc.vector.tensor_scalar_add(
                    dst[:, m, ts], pq[:, 0:512], b_pp[:, m : m + 1]
                )

            def proj_v(u):
                """v for key tiles 2u, 2u+1 -> v_sb.  The two tiles go to
                different PSUM banks of one ring tile with their kk chains
                interleaved, so adjacent matmuls are independent."""
                pv = ring.tile([P, TW], F32, tag="big")
                regions = (slice(0, G), slice(512, 512 + G))
                for kk in range(KT):
                    for i in range(2):
                        nc.tensor.matmul(
                            pv[:, regions[i]],
                            xs[:, kk, (2 * u + i) * P : (2 * u + i + 1) * P],
                            wv_sb[:, kk, :],
                            start=(kk == 0),
                            stop=False,
                            skip_group_check=True,
                        )
                for i in range(2):
                    nc.tensor.matmul(
                        pv[:, regions[i]], ones_col[0:1, :], bv_row[0:1, :],
                        start=False, stop=True,
                        skip_group_check=True,
                    )
                for i in range(2):
                    nc.vector.tensor_copy(
                        v_sb[:, 2 * u + i, :, 0:D],
                        pv[:, regions[i]].rearrange("p (h d) -> p h d", d=D),
                    )

            def oproj(mt, act_copy=False):
                po = ring.tile([P, TW], F32, tag="big", name="po")
                for j in range(2):  # halves interleaved: independent banks
                    for n in range(2):
                        ns = slice(n * 512, (n + 1) * 512)
                        nc.tensor.matmul(
                            po[:, ns],
                            yT[:, j, mt * P : (mt + 1) * P],
                            wp_sb[:, j, ns],
                            start=(j == 0),
                            stop=(j == 1),
                        )
                ob = obuf.tile([P, C], BF16, tag="ob")
                for n in range(2):  # halves: DMA overlaps the second copy;
                    # in the tail the exp stream is done, so ACT takes one
                    # half end-to-end (its own copy AND its own DMA channel)
                    ns = slice(n * 512, (n + 1) * 512)
                    if act_copy and n == 0:
                        nc.scalar.copy(ob[:, ns], po[:, ns])
                        nc.scalar.dma_start(
                            out[mt * P : (mt + 1) * P, ns], ob[:, ns]
                        )
                    else:
                        nc.vector.tensor_copy(ob[:, ns], po[:, ns])
                        nc.sync.dma_start(
                            out[mt * P : (mt + 1) * P, ns], ob[:, ns]
                        )

            def s_mm(pi, tq, s):
                """S scores for key tile s, both heads of pair pi (quadrant
                pair: base partitions 0 and 64 run concurrently on the PE)."""
                ts = slice(tq * TQ, (tq + 1) * TQ)
                sp = ring.tile([P, 2 * TQ], F32, tag="big")
                for hh in range(2):
                    bp_ = 64 * hh
                    nc.tensor.matmul(
                        sp[:, hh * TQ : (hh + 1) * TQ],
                        kT[bp_ : bp_ + 64, pi, s * P : (s + 1) * P],
                        qT[bp_ : bp_ + 64, pi, ts],
                        start=True,
                        stop=True,
                    )
                # P = exp(S/8 - ln16) in bf16 (the shift cancels in y'/sigma)
                pt = ptp.tile([P, 2, TQ], BF16, tag="pt")
                nc.scalar.activation(
                    pt[:], sp[:], Act.Exp, bias=expb[:, 0:1], scale=0.125,
                )
                return pt

            def normalize_tail(pi, tq):
                """Last chunk: normalize in 128-column blocks and launch each
                out-projection as soon as its block of yT is ready, so the PE
                restarts early and the copies/DMAs pipeline behind it."""
                ts = slice(tq * TQ, (tq + 1) * TQ)
                y_uns, bcasts = [], []
                for hh in range(2):
                    py = (py0_ref[0], py1_ref[0])[hh]
                    y_un = npool.tile([VW, TQ], F32, tag=f"yun{hh}")
                    nc.vector.tensor_copy(y_un[:], py[:])
                    y_uns.append(y_un)
                for hh in range(2):
                    rec0 = npool.tile([1, TQ], F32, tag=f"rec0{hh}")
                    nc.sync.dma_start(rec0[:], y_uns[hh][D : D + 1, :])
                    rinv = npool.tile([1, TQ], F32, tag=f"rinv{hh}")
                    nc.vector.reciprocal_approx_fast(rinv[0:1, :], rec0[0:1, :])
                    bcast = npool.tile([D, TQ], F32, tag=f"bcast{hh}")
                    nc.gpsimd.partition_broadcast(
                        bcast[:, :], rinv[0:1, :], channels=D
                    )
                    bcasts.append(bcast)
                y_tmp = npool.tile([D, TQ], BF16, tag="y_tmp")
                for w in range(4):
                    bs = slice(w * P, (w + 1) * P)
                    tbs = slice(tq * TQ + w * P, tq * TQ + (w + 1) * P)
                    nc.vector.tensor_mul(
                        yT[0:D, pi, tbs], y_uns[0][0:D, bs], bcasts[0][:, bs]
                    )
                    nc.vector.tensor_mul(
                        y_tmp[:, bs], y_uns[1][0:D, bs], bcasts[1][:, bs]
                    )
                    nc.sync.dma_start(yT[D : 2 * D, pi, tbs], y_tmp[:, bs])
                    oproj(tq * 4 + w, act_copy=True)

            def normalize(pi, tq):
                """y_h <- y_h / sigma_h for both heads of pair pi, query
                chunk tq.  Copies py to SBUF first (releases the PSUM bank
                ~1us after the s-loop), then the slow part runs on
                DVE/Pool/DMA off the PE critical path."""
                ts = slice(tq * TQ, (tq + 1) * TQ)
                y_uns = []
                for hh in range(2):  # both copies first: frees py banks fast
                    py = (py0_ref[0], py1_ref[0])[hh]
                    y_un = npool.tile([VW, TQ], F32, tag=f"yun{hh}")
                    nc.vector.tensor_copy(y_un[:], py[:])
                    y_uns.append(y_un)
                for hh in range(2):
                    y_un = y_uns[hh]
                    # sigma to partition 0 (reciprocal_approx_fast is only
                    # correct at base partition 0), approx 1/sigma, broadcast
                    rec0 = npool.tile([1, TQ], F32, tag=f"rec0{hh}")
                    nc.sync.dma_start(rec0[:], y_un[D : D + 1, :])
                    rinv = npool.tile([1, TQ], F32, tag=f"rinv{hh}")
                    nc.vector.reciprocal_approx_fast(rinv[0:1, :], rec0[0:1, :])
                    bcast = npool.tile([D, TQ], F32, tag=f"bcast{hh}")
                    nc.gpsimd.partition_broadcast(
                        bcast[:, :], rinv[0:1, :], channels=D
                    )
                    if hh == 0:
                        nc.vector.tensor_mul(
                            yT[0:D, pi, ts], y_un[0:D, :], bcast[:, :]
                        )
                    else:
                        y_tmp = npool.tile([D, TQ], BF16, tag="y_tmp")
                        nc.vector.tensor_mul(y_tmp[:], y_un[0:D, :], bcast[:, :])
                        nc.sync.dma_start(yT[D : 2 * D, pi, ts], y_tmp[:])

            # JIT emission schedule: extras[(pi, tq, s)] = thunks run at the
            # top of that attention s iteration (PE program order).
            extras = {}
            # v lumps lead their Y consumers by >=2 iterations; v(0) is NOT
            # in the prologue — its wv/x DMAs land ~10us in, and emitting it
            # before S(0) would stall the in-order PE queue on the DMA
            vsched = {0: 1, 1: 2, 2: 3, 3: 4, 4: 6, 5: 8, 6: 10, 7: 12}
            for u, key in vsched.items():
                extras.setdefault((0, 0, key), []).append(
                    lambda u=u: proj_v(u)
                )
            # second halves of k/q chunk 0 (prologue did only t 0:512)
            extras.setdefault((0, 0, 1), []).append(
                lambda: proj_qk_half(0, 0, 1, 1)
            )
            extras.setdefault((0, 0, 5), []).append(
                lambda: proj_qk_half(0, 0, 1, 0)
            )
            extras.setdefault((0, 0, 3), []).append(lambda: proj_qk(0, 1, 1))
            extras.setdefault((0, 0, 11), []).append(lambda: proj_qk(0, 1, 0))
            extras.setdefault((0, 1, 0), []).append(lambda: proj_qk(1, 0, 1))
            extras.setdefault((0, 1, 5), []).append(lambda: proj_qk(1, 1, 1))
            extras.setdefault((0, 1, 10), []).append(lambda: proj_qk(1, 0, 0))
            extras.setdefault((0, 2, 0), []).append(lambda: proj_qk(1, 1, 0))
            for tq in range(1, NTQ):  # deferred out-projection of tq-1
                for w, key in enumerate((6, 9, 11, 13)):
                    extras.setdefault((1, tq, key), []).append(
                        lambda tq=tq, w=w: oproj((tq - 1) * 4 + w)
                    )

            # minimal prologue: k and q for t 0:512 only — the s-loop (and
            # with it the ACT exp stream) starts as soon as those project
            proj_qk_half(0, 0, 0, 1)
            proj_qk_half(0, 0, 0, 0)

            py0_ref = [None]
            py1_ref = [None]

            for pi in range(2):
                for tq in range(NTQ):
                    py0_ref[0] = psY.tile(
                        [VW, TQ], F32, tag="py0", name="py0"
                    )
                    py1_ref[0] = psY.tile(
                        [VW, TQ], F32, tag="py1", name="py1"
                    )
                    pts = {}
                    for j in range(2):  # depth-2 S prefetch: two exp tiles
                        for fn in extras.get((pi, tq, j), ()):  # in flight
                            fn()
                        pts[j] = s_mm(pi, tq, j)
                    for s in range(NS):
                        if s + 2 < NS:
                            for fn in extras.get((pi, tq, s + 2), ()):
                                fn()
                            pts[s + 2] = s_mm(pi, tq, s + 2)
                        pt = pts.pop(s)
                        for hh in range(2):
                            h = 2 * pi + hh
                            nc.tensor.matmul(
                                (py0_ref[0], py1_ref[0])[hh][:],
                                v_sb[:, s, h, :],
                                pt[:, hh, :],
                                start=(s == 0),
                                stop=(s == NS - 1),
                            )
                    if pi == 1 and tq == NTQ - 1:
                        normalize_tail(pi, tq)
                    else:
                        normalize(pi, tq)

    nc.finalize()
    return nc


_NC_CACHE = {}


def _get_nc(T=2048):
    if T not in _NC_CACHE:
        _NC_CACHE[T] = build(T=T)
    return _NC_CACHE[T]


def _make_in_maps(x, Wq, bq, Wk, bk, Wv, bv, Wp):
    in_maps = []
    for b in range(B):
        xt = np.ascontiguousarray(x[b].T).astype(NP_BF16)
        for g in range(GROUPS):
            sl = slice(g * G, (g + 1) * G)
            in_maps.append(
                {
                    "xt": xt,
                    "wq": np.ascontiguousarray(Wq[sl, :].T).astype(NP_BF16),
                    "wk": np.ascontiguousarray(Wk[sl, :].T).astype(NP_BF16),
                    "wv": np.ascontiguousarray(Wv[sl, :].T).astype(NP_BF16),
                    "wp": np.ascontiguousarray(Wp[:, sl].T).astype(NP_BF16),
                    "bq": np.ascontiguousarray(bq[sl], dtype=np.float32),
                    "bk": np.ascontiguousarray(bk[sl], dtype=np.float32),
                    "bv": np.ascontiguousarray(bv[sl]).astype(NP_BF16),
                }
            )
    return in_maps


def run(inputs, trace=False):
    """Run on 8 cores; returns (out [B,T,C] fp32, BassKernelResults)."""
    x = np.asarray(inputs["x"], dtype=np.float32)
    T = x.shape[1]
    in_maps = _make_in_maps(
        x,
        np.asarray(inputs["Wq"]), np.asarray(inputs["bq"]),
        np.asarray(inputs["Wk"]), np.asarray(inputs["bk"]),
        np.asarray(inputs["Wv"]), np.asarray(inputs["bv"]),
        np.asarray(inputs["Wp"]),
    )
    nc = _get_nc(T)
    res = run_bass_kernel_spmd(
        nc, in_maps, core_ids=list(range(B * GROUPS)), trace=trace
    )
    bp = np.asarray(inputs["bp"], dtype=np.float32)
    parts = [
        res.results[i]["out"].astype(np.float32) for i in range(B * GROUPS)
    ]
    out = np.stack(
        [sum(parts[b * GROUPS : (b + 1) * GROUPS]) for b in range(B)]
    ) + bp[None, None, :]
    return out.astype(np.float32), res


def kernel(**inputs):
    out, _ = run(inputs, trace=False)
    return out
